# revision 28
# baseline (speedup 1.0000x reference)
"""Masked self-attention Trainium2 kernel (8 NeuronCores, Bass/Tile).

Problem: B=4, S=2048, D=1024, DK=128 fp32.
  Q = X@Wq + bq; K = X@Wk + bk; V = X@Wv + bv
  scores = Q@K^T / sqrt(DK); masked = scores + tril(ones)*(-1e9)
  out = softmax(masked) @ V

Sharding: core = (batch b = core//2) x (row-half h = core%2). Each core
computes 64 query rows of each of the 16 query tiles of its batch
(rows 128c + 64h + j). All cores run an identical program; per-core
differences are carried entirely in the input data (a column
permutation of X^T and a small mask block).

Device layouts (all transposed so the PE contracts over partitions):
  X^T [D, S] (host-transposed, per-tile column permuted: own rows first)
  Q^T/K^T [DK, *] = W-chunks(lhsT) x X^T(moving) matmuls
  scores^T [s-chunk 128, q-prefix] = K^T-chunk(lhsT) x Q^T(moving)
  causal skip: chunk c only attends query tiles qi <= c -> contiguous
  q-prefix of width 64*(c+1); single [128,64] mask block on the last
  64 columns (the diagonal tile)
  softmax: exp without max-subtraction (scores are O(1); masked lanes
  underflow to exactly 0); row sums via an all-ones [128,128] stationary
  matmul that lands the sums REPLICATED across all 128 partitions (same
  PE cost as an M=1 matmul -- cost is moving columns), so normalization
  is a single DVE fast-reciprocal + multiply. No ScalarE ln/exp tail,
  no activation-table switches (Exp is the only table ever loaded).
  out^T [DK, 1024] accumulated in PSUM across s-chunks.
  The globally fully-masked last row (2047) is patched on the HOST:
  softmax of a uniform -1e9 row is exactly uniform, so that row equals
  mean_s(V) = mean_s(X) @ Wv + bv -- a [D]x[D,DK] matvec on numpy.
  (On-device that column is 0/0 -> NaN and is simply overwritten.)

  All matmul operands are float16 (11-bit mantissa, ~2.4e-4 rounding --
  the same precision class as the PE's f32r/TF32 mode for this N(0,1)
  data) with fp32 PSUM accumulation. vs f32r this halves the X DMA,
  enables fast weight loads (FWL; fp32-path LDWEIGHTS cannot be
  hidden), and has no small-N throughput penalty. Range is safe: all
  fp16-stored tensors are O(1)..O(100); scores/sums/outputs stay fp32.
  The first weight chunk gets a dedicated small first-wave DMA because
  the DGE queues fair-share HBM bandwidth and gate the first matmul;
  block 0's X arrives in fine-grained pieces (dc0 first) so the PE
  starts as soon as the first 192 KiB have landed.
"""

import numpy as np

import concourse.bacc as bacc
import concourse.tile as tile
import concourse.mybir as mybir
from concourse.bass_utils import run_bass_kernel_spmd

F32 = mybir.dt.float32
F16 = mybir.dt.float16
AF = mybir.ActivationFunctionType

B, S, D, DK = 4, 2048, 1024, 128
NEG = -1.0e9
NCORES = 8
NBLK = 4          # s-blocks of 512
NCHUNK = 16       # s-chunks of 128
QL = 1024         # local query columns per core (16 tiles x 64)

_cache = {}


def _build():
    nc = bacc.Bacc("TRN2", target_bir_lowering=False, debug=False,
                   num_devices=NCORES)

    xt = nc.dram_tensor("xt", [D, S], F16, kind="ExternalInput")
    wq = nc.dram_tensor("wq", [128, 8, DK], F16, kind="ExternalInput")
    wk = nc.dram_tensor("wk", [128, 8, DK], F16, kind="ExternalInput")
    wv = nc.dram_tensor("wv", [128, 8, DK], F16, kind="ExternalInput")
    bq = nc.dram_tensor("bq", [DK, 1], F32, kind="ExternalInput")
    bk = nc.dram_tensor("bk", [DK, 1], F32, kind="ExternalInput")
    bv = nc.dram_tensor("bv", [DK, 1], F32, kind="ExternalInput")
    maskd = nc.dram_tensor("maskd", [128, 64], F32, kind="ExternalInput")
    onesd = nc.dram_tensor("onesd", [128, 128], F16, kind="ExternalInput")
    idend = nc.dram_tensor("idend", [128, 128], F16, kind="ExternalInput")
    outT = nc.dram_tensor("outT", [DK, QL], F16, kind="ExternalOutput")

    with tile.TileContext(nc) as tc:
        with (
            tc.tile_pool(name="consts", bufs=1) as cpool,
            tc.tile_pool(name="xblk", bufs=3) as xpool,
            tc.tile_pool(name="kv", bufs=1) as kvpool,
            tc.tile_pool(name="pt", bufs=6) as ppool,
            tc.tile_pool(name="outp", bufs=1) as opool,
            tc.tile_pool(name="ps_out", bufs=1, space="PSUM") as ps_out_pool,
            tc.tile_pool(name="ps_sums", bufs=1, space="PSUM") as ps_sums_pool,
            tc.tile_pool(name="ps_proj", bufs=2, space="PSUM") as ps_proj_pool,
            tc.tile_pool(name="ps_score", bufs=2, space="PSUM") as ps_score_pool,
        ):
            # ---- weights first (needed by the very first matmul).
            # The first proj matmul (K, dc=0) gates the whole PE stream, so
            # its 64 KiB weight chunk gets a dedicated first DMA: the DGE
            # queues fair-share HBM bandwidth, so a small exclusive first
            # wave completes ~10x sooner than one queued with everything.
            w_sb = {}
            for name, dram in (("k", wk), ("v", wv), ("q", wq)):
                t = cpool.tile([128, 8, DK], F16, tag=f"w{name}")
            # Weights ride the scalar ring in need-order (wk, wv, wq) so
            # they serialize behind each other instead of splitting HBM
            # bandwidth with X block 0; two X pieces are interleaved into
            # the ring so X keeps a larger share of the early window.
            xb0 = xpool.tile([128, 8, 512], F16, tag="xb")
            for name, dram in (("k", wk), ("v", wv), ("q", wq)):
                t = cpool.tile([128, 8, DK], F16, tag=f"w{name}")
                if name == "k":
                    nc.scalar.dma_start(out=t[:, 0:1], in_=dram[:, 0:1])
                    nc.scalar.dma_start(out=xb0[:, 1:2], in_=xt[128:256, 0:512])
                    nc.scalar.dma_start(out=t[:, 1:8], in_=dram[:, 1:8])
                    nc.scalar.dma_start(out=xb0[:, 3:4], in_=xt[384:512, 0:512])
                else:
                    nc.scalar.dma_start(out=t[:], in_=dram[:])
                w_sb[name] = t

            def small_consts():
                b_sb = {}
                for name, dram in (("q", bq), ("k", bk), ("v", bv)):
                    t = cpool.tile([DK, 1], F32, tag=f"b{name}")
                    nc.gpsimd.dma_start(out=t[:], in_=dram[:])
                    b_sb[name] = t
                mask_sb = cpool.tile([128, 64], F32, tag="mask")
                nc.gpsimd.dma_start(out=mask_sb[:], in_=maskd[:])
                ones_sb = cpool.tile([128, 128], F16, tag="ones")
                nc.gpsimd.dma_start(out=ones_sb[:], in_=onesd[:])
                iden_sb = cpool.tile([128, 128], F16, tag="iden")
                nc.gpsimd.dma_start(out=iden_sb[:], in_=idend[:])
                return b_sb, mask_sb, ones_sb, iden_sb

            # ---- persistent buffers ----
            kT_sb = kvpool.tile([DK, S], F16, tag="kT")
            qT_sb = kvpool.tile([DK, QL], F16, tag="qT")
            vT_sb = kvpool.tile([DK, S], F16, tag="vT")
            vnat_sb = kvpool.tile([128, NCHUNK, DK], F16, tag="vnat")

            ps_out = ps_out_pool.tile([DK, QL], F32)       # 2 banks
            ps_sums = ps_sums_pool.tile([128, QL], F32)    # 2 banks
            nc.vector.memset(ps_out[:], 0.0)
            nc.vector.memset(ps_sums[:], 0.0)

            for blk in range(NBLK):
                s0 = blk * 512
                # ---- stream X^T block: 8 d-chunk tiles x 512 s-cols ----
                # Block 0 lands in fine pieces (dc0 alone first: together
                # with the wk first wave only ~192 KiB gate the first
                # matmul); later blocks use coarse 2-chunk DMAs to save
                # issue slots on the queues.
                if blk == 0:
                    # single-d-chunk pieces: the PE stalls on each 128 KiB
                    # piece individually instead of a 256 KiB pair while
                    # the early phase is HBM-bound (dc1/dc3 already ride
                    # the scalar ring between the weight DMAs)
                    xb = xb0
                    for dc in (0, 2, 4, 5, 6, 7):
                        nc.sync.dma_start(out=xb[:, dc:dc + 1],
                                          in_=xt[128 * dc:128 * dc + 128, 0:512])
                else:
                    xb = xpool.tile([128, 8, 512], F16, tag="xb")
                    for dc in range(2):
                        nc.sync.dma_start(
                            out=xb[:, 4 * dc:4 * dc + 4],
                            in_=xt[512 * dc:512 * dc + 512, s0:s0 + 512]
                            .rearrange("(i p) s -> p i s", p=128),
                        )
                if blk == 0:
                    b_sb, mask_sb, ones_sb, iden_sb = small_consts()

                # ---- K^T / V^T projections for this block ----
                for name, dst in (("k", kT_sb), ("v", vT_sb)):
                    pp = ps_proj_pool.tile([DK, 512], F32, tag="pp")
                    for dc in range(8):
                        nc.tensor.matmul(
                            pp[:], w_sb[name][:, dc], xb[:, dc],
                            start=(dc == 0), stop=(dc == 7),
                        )
                    nc.vector.tensor_scalar_add(
                        dst[:, s0:s0 + 512], pp[:], b_sb[name][:],
                    )

                # ---- Q^T projection: first 64 cols of each 128-tile ----
                pq = ps_proj_pool.tile([DK, 256], F32, tag="pp")
                for dc in range(8):
                    qmov = xb[:, dc].rearrange("p (t j) -> p t j", t=4)[:, :, 0:64]
                    nc.tensor.matmul(
                        pq[:], w_sb["q"][:, dc], qmov,
                        start=(dc == 0), stop=(dc == 7),
                    )
                q0 = blk * 256
                nc.vector.tensor_scalar_add(qT_sb[:, q0:q0 + 256], pq[:], b_sb["q"][:])

                # ---- V natural tiles (transpose V^T chunks) ----
                tp4 = ps_proj_pool.tile([128, 4, 128], F16, tag="pp")
                for t in range(4):
                    c = 4 * blk + t
                    nc.tensor.matmul(
                        tp4[:, t], vT_sb[:, 128 * c:128 * c + 128], iden_sb[:],
                        is_transpose=True, start=(t == 0), stop=(t == 3),
                    )
                nc.vector.tensor_copy(vnat_sb[:, 4 * blk:4 * blk + 4], tp4[:])

                # ---- attention chunks for this block ----
                # last block reversed: the big chunk 15 goes first so the
                # final chunk on the critical path is the smaller chunk 12;
                # chunk 12's pieces run high-half first so the output's
                # [512:1024] half can normalize + DMA while the PE finishes
                # the [0:512) piece.
                order = range(3, -1, -1) if blk == NBLK - 1 else range(4)
                for t in order:
                    c = 4 * blk + t
                    prefix = 64 * (c + 1)
                    dcol = 64 * c  # diagonal columns [dcol, dcol+64)
                    # 256-col pieces: the PE's wait for the first exp of a
                    # chunk shrinks to ~exp(256), and the 4-deep sc/pt
                    # rotation (same PSUM/SBUF budget) keeps scores running
                    # ahead of the exps
                    pieces = [(p, min(256, prefix - p))
                              for p in range(0, prefix, 256)]
                    if c == 12:
                        pieces.reverse()
                    kT_c = kT_sb[:, 128 * c:128 * c + 128]

                    def norm_store(lo, hi):
                        # this column range of the output is final:
                        # normalize + store it while the PE moves on.
                        # ps_sums holds the denominators replicated across
                        # all 128 partitions, so this is pure elementwise
                        # work; the fully-masked column is 0 -> NaN and
                        # the host overwrites it.
                        recip = opool.tile([128, hi - lo], F32,
                                           tag=f"recip{lo}")
                        nc.vector.reciprocal_approx_fast(
                            recip[:], ps_sums[:, lo:hi])
                        o_h = opool.tile([DK, hi - lo], F16, tag=f"o{lo}")
                        nc.vector.tensor_tensor(
                            o_h[:], ps_out[:, lo:hi],
                            recip[:], mybir.AluOpType.mult)
                        nc.sync.dma_start(out=outT[:, lo:hi], in_=o_h[:])
                    # all scores matmuls first (back-to-back on the PE), so
                    # ScalarE runs the exps back-to-back too and exp(p2)
                    # overlaps AV/sums(p1) instead of trailing them
                    pts = []
                    sc2 = None
                    for idx, (p0, pn) in enumerate(pieces):
                        # two 256-col pieces share one bank-sized PSUM tile
                        # (PSUM pool buffers are bank-granular), giving a
                        # 4-deep piece rotation in 2 banks
                        if idx % 2 == 0:
                            sc2 = ps_score_pool.tile([128, 2, 256], F32,
                                                     tag="sc")
                        sc = sc2[:, idx % 2]
                        nc.tensor.matmul(
                            sc[:, 0:pn], kT_c, qT_sb[:, p0:p0 + pn],
                            start=True, stop=True,
                        )
                        if p0 <= dcol < p0 + pn:
                            dl = dcol - p0
                            nc.vector.tensor_tensor(
                                sc[:, dl:dl + 64], sc[:, dl:dl + 64],
                                mask_sb[:], mybir.AluOpType.add,
                            )
                        pt = ppool.tile([128, 256], F16, tag="pt")
                        nc.scalar.activation(pt[:, 0:pn], sc[:, 0:pn], AF.Exp)
                        pts.append(pt)
                    if c == 12:
                        # chunks 15, 14, 13 are done and chunk 12 only
                        # touches [0, 832): columns [832:1024) are final.
                        # Emitted after chunk 12's mask add so the DVE work
                        # here doesn't delay its exp.
                        norm_store(832, 1024)
                    for (p0, pn), pt in zip(pieces, pts):
                        # the accumulators were DVE-zeroed once up front, so
                        # every matmul accumulates (start=False)
                        nc.tensor.matmul(
                            ps_out[:, p0:p0 + pn], vnat_sb[:, c],
                            pt[:, 0:pn], start=False, stop=False,
                        )
                        nc.tensor.matmul(
                            ps_sums[:, p0:p0 + pn], ones_sb[:],
                            pt[:, 0:pn], start=False, stop=False,
                        )
                        if c == 12 and p0 in (0, 256, 512):
                            # pieces run reversed, so [p0:832) is complete
                            # once this piece's accumulation lands; the
                            # p0=512 range spans the (768,64) runt piece
                            norm_store(p0, 832 if p0 == 512 else p0 + 256)

    nc.compile()
    return nc


def _prep_inputs(inputs, Wq, bq, Wk, bk, Wv, bv):
    scale = np.float32(1.0 / np.sqrt(DK))
    wq_s = np.ascontiguousarray((Wq * scale).reshape(8, 128, DK).transpose(1, 0, 2)).astype(np.float16)
    wk_s = np.ascontiguousarray(Wk.reshape(8, 128, DK).transpose(1, 0, 2)).astype(np.float16)
    wv_s = np.ascontiguousarray(Wv.reshape(8, 128, DK).transpose(1, 0, 2)).astype(np.float16)
    bq_s = np.ascontiguousarray((bq * scale).reshape(DK, 1), dtype=np.float32)
    bk_s = np.ascontiguousarray(bk.reshape(DK, 1), dtype=np.float32)
    bv_s = np.ascontiguousarray(bv.reshape(DK, 1), dtype=np.float32)
    ones = np.ones((128, 128), dtype=np.float16)
    iden = np.eye(128, dtype=np.float16)

    p = np.arange(128)[:, None]
    j = np.arange(64)[None, :]
    masks = []
    for h in (0, 1):
        m = np.zeros((128, 64), dtype=np.float32)
        m[(p < 64) & (p <= j)] = NEG
        if h == 1:
            m[p[:, 0] >= 64, :] = NEG
        masks.append(m)

    in_maps = []
    for core in range(NCORES):
        b, h = core // 2, core % 2
        xt = inputs[b].T.reshape(D, 16, 2, 64)
        if h == 1:
            xt = xt[:, :, ::-1, :]
        xt = np.ascontiguousarray(xt).reshape(D, S).astype(np.float16)
        in_maps.append({
            "xt": xt, "wq": wq_s, "wk": wk_s, "wv": wv_s,
            "bq": bq_s, "bk": bk_s, "bv": bv_s,
            "maskd": masks[h], "onesd": ones, "idend": iden,
        })
    return in_maps


def kernel(inputs, Wq, bq, Wk, bk, Wv, bv):
    inputs = np.asarray(inputs, dtype=np.float32)
    Wq, bq = np.asarray(Wq), np.asarray(bq)
    Wk, bk = np.asarray(Wk), np.asarray(bk)
    Wv, bv = np.asarray(Wv), np.asarray(bv)
    if "nc" not in _cache:
        _cache["nc"] = _build()
    nc = _cache["nc"]
    in_maps = _prep_inputs(inputs, Wq, bq, Wk, bk, Wv, bv)
    res = run_bass_kernel_spmd(nc, in_maps, list(range(NCORES)))
    out = np.empty((B, S, DK), dtype=np.float32)
    for core in range(NCORES):
        b, h = core // 2, core % 2
        oT = res.results[core]["outT"].astype(np.float32)  # [DK, 1024]
        o = oT.T.reshape(16, 64, DK)            # [c, j, DK]
        out[b].reshape(16, 2, 64, DK)[:, h] = o
    # Fully-masked last row: softmax of a uniform -1e9 row is exactly
    # uniform, so out[2047] = mean_s(V) = mean_s(X) @ Wv + bv.
    xmean = inputs.mean(axis=1)                 # [B, D]
    out[:, S - 1, :] = xmean @ Wv + bv
    return out


# revision 34
# speedup vs baseline: 1.0722x; 1.0722x over previous
"""Masked self-attention Trainium2 kernel (8 NeuronCores, Bass/Tile).

Problem: B=4, S=2048, D=1024, DK=128 fp32.
  Q = X@Wq + bq; K = X@Wk + bk; V = X@Wv + bv
  scores = Q@K^T / sqrt(DK); masked = scores + tril(ones)*(-1e9)
  out = softmax(masked) @ V

Sharding: core = (batch b = core//2) x (row-half h = core%2). Each core
computes 64 query rows of each of the 16 query tiles of its batch
(rows 128c + 64h + j). All cores run an identical program; per-core
differences are carried entirely in the input data (a column
permutation of X^T and a small mask block).

Device layouts (all transposed so the PE contracts over partitions):
  X^T [D, S] (host-transposed, per-tile column permuted: own rows first)
  Q^T/K^T [DK, *] = W-chunks(lhsT) x X^T(moving) matmuls
  scores^T [s-chunk 128, q-prefix] = K^T-chunk(lhsT) x Q^T(moving)
  causal skip: chunk c only attends query tiles qi <= c -> contiguous
  q-prefix of width 64*(c+1); single [128,64] mask block on the last
  64 columns (the diagonal tile)
  softmax: exp without max-subtraction (scores are O(1); masked lanes
  underflow to exactly 0); row sums via an all-ones [128,128] stationary
  matmul that lands the sums REPLICATED across all 128 partitions (same
  PE cost as an M=1 matmul -- cost is moving columns), so normalization
  is a single DVE fast-reciprocal + multiply. No ScalarE ln/exp tail,
  no activation-table switches (Exp is the only table ever loaded).
  out^T [DK, 1024] accumulated in PSUM across s-chunks.
  The globally fully-masked last row (2047) is patched on the HOST:
  softmax of a uniform -1e9 row is exactly uniform, so that row equals
  mean_s(V) = mean_s(X) @ Wv + bv -- a [D]x[D,DK] matvec on numpy.
  (On-device that column is 0/0 -> NaN and is simply overwritten.)

  All matmul operands are float16 (11-bit mantissa, ~2.4e-4 rounding --
  the same precision class as the PE's f32r/TF32 mode for this N(0,1)
  data) with fp32 PSUM accumulation. vs f32r this halves the X DMA,
  enables fast weight loads (FWL; fp32-path LDWEIGHTS cannot be
  hidden), and has no small-N throughput penalty. Range is safe: all
  fp16-stored tensors are O(1)..O(100); scores/sums/outputs stay fp32.
  The first weight chunk gets a dedicated small first-wave DMA because
  the DGE queues fair-share HBM bandwidth and gate the first matmul;
  block 0's X arrives in fine-grained pieces (dc0 first) so the PE
  starts as soon as the first 192 KiB have landed.
"""

import numpy as np

import concourse.bacc as bacc
import concourse.tile as tile
import concourse.mybir as mybir
from concourse.bass_utils import run_bass_kernel_spmd

F32 = mybir.dt.float32
F16 = mybir.dt.float16
AF = mybir.ActivationFunctionType

B, S, D, DK = 4, 2048, 1024, 128
NEG = -1.0e9
NCORES = 8
NBLK = 4          # s-blocks of 512
NCHUNK = 16       # s-chunks of 128
QL = 1024         # local query columns per core (16 tiles x 64)

_cache = {}


def _build():
    nc = bacc.Bacc("TRN2", target_bir_lowering=False, debug=False,
                   num_devices=NCORES)

    xt = nc.dram_tensor("xt", [D, S], F16, kind="ExternalInput")
    wq = nc.dram_tensor("wq", [128, 8, DK], F16, kind="ExternalInput")
    wk = nc.dram_tensor("wk", [128, 8, DK], F16, kind="ExternalInput")
    wv = nc.dram_tensor("wv", [128, 8, DK], F16, kind="ExternalInput")
    bq = nc.dram_tensor("bq", [DK, 1], F32, kind="ExternalInput")
    bk = nc.dram_tensor("bk", [DK, 1], F32, kind="ExternalInput")
    bv = nc.dram_tensor("bv", [DK, 1], F32, kind="ExternalInput")
    maskd = nc.dram_tensor("maskd", [128, 64], F32, kind="ExternalInput")
    onesd = nc.dram_tensor("onesd", [128, 128], F16, kind="ExternalInput")
    idend = nc.dram_tensor("idend", [128, 128], F16, kind="ExternalInput")
    outT = nc.dram_tensor("outT", [DK, QL], F16, kind="ExternalOutput")

    with tile.TileContext(nc) as tc:
        with (
            tc.tile_pool(name="consts", bufs=1) as cpool,
            tc.tile_pool(name="xblk", bufs=3) as xpool,
            tc.tile_pool(name="kv", bufs=1) as kvpool,
            tc.tile_pool(name="pt", bufs=3) as ppool,
            tc.tile_pool(name="outp", bufs=1) as opool,
            tc.tile_pool(name="ps_out", bufs=1, space="PSUM") as ps_out_pool,
            tc.tile_pool(name="ps_sums", bufs=1, space="PSUM") as ps_sums_pool,
            tc.tile_pool(name="ps_proj", bufs=2, space="PSUM") as ps_proj_pool,
            tc.tile_pool(name="ps_score", bufs=2, space="PSUM") as ps_score_pool,
        ):
            # ---- weights first (needed by the very first matmul).
            # The first proj matmul (K, dc=0) gates the whole PE stream, so
            # its 64 KiB weight chunk gets a dedicated first DMA: the DGE
            # queues fair-share HBM bandwidth, so a small exclusive first
            # wave completes ~10x sooner than one queued with everything.
            # Weights ride the scalar ring in need-order (wk, wv, wq) so
            # they serialize behind each other instead of splitting HBM
            # bandwidth with X block 0; two X pieces are interleaved into
            # the ring so X keeps a larger share of the early window.
            w_sb = {}
            xb0 = xpool.tile([128, 8, 512], F16, tag="xb")
            for name, dram in (("k", wk), ("v", wv), ("q", wq)):
                t = cpool.tile([128, 8, DK], F16, tag=f"w{name}")
                if name == "k":
                    nc.scalar.dma_start(out=t[:, 0:1], in_=dram[:, 0:1])
                    nc.scalar.dma_start(out=xb0[:, 1:2], in_=xt[128:256, 0:512])
                    nc.scalar.dma_start(out=t[:, 1:8], in_=dram[:, 1:8])
                    nc.scalar.dma_start(out=xb0[:, 3:4], in_=xt[384:512, 0:512])
                else:
                    nc.scalar.dma_start(out=t[:], in_=dram[:])
                w_sb[name] = t

            def small_consts():
                b_sb = {}
                for name, dram in (("q", bq), ("k", bk), ("v", bv)):
                    t = cpool.tile([DK, 1], F32, tag=f"b{name}")
                    nc.gpsimd.dma_start(out=t[:], in_=dram[:])
                    b_sb[name] = t
                mask_sb = cpool.tile([128, 64], F32, tag="mask")
                nc.gpsimd.dma_start(out=mask_sb[:], in_=maskd[:])
                ones_sb = cpool.tile([128, 128], F16, tag="ones")
                nc.gpsimd.dma_start(out=ones_sb[:], in_=onesd[:])
                iden_sb = cpool.tile([128, 128], F16, tag="iden")
                nc.gpsimd.dma_start(out=iden_sb[:], in_=idend[:])
                return b_sb, mask_sb, ones_sb, iden_sb

            # ---- persistent buffers ----
            kT_sb = kvpool.tile([DK, S], F16, tag="kT")
            qT_sb = kvpool.tile([DK, QL], F16, tag="qT")
            vT_sb = kvpool.tile([DK, S], F16, tag="vT")
            vnat_sb = kvpool.tile([128, NCHUNK, DK], F16, tag="vnat")

            ps_out = ps_out_pool.tile([DK, QL], F32)       # 2 banks
            ps_sums = ps_sums_pool.tile([128, QL], F32)    # 2 banks
            nc.vector.memset(ps_out[:], 0.0)
            nc.vector.memset(ps_sums[:], 0.0)

            for blk in range(NBLK):
                s0 = blk * 512
                # ---- stream X^T block: 8 d-chunk tiles x 512 s-cols ----
                # Block 0 lands in fine pieces (dc0 alone first: together
                # with the wk first wave only ~192 KiB gate the first
                # matmul); later blocks use coarse 2-chunk DMAs to save
                # issue slots on the queues.
                if blk == 0:
                    # single-d-chunk pieces: the PE stalls on each 128 KiB
                    # piece individually instead of a 256 KiB pair while
                    # the early phase is HBM-bound (dc1/dc3 already ride
                    # the scalar ring between the weight DMAs)
                    xb = xb0
                    for dc in (0, 2, 4, 5, 6, 7):
                        nc.sync.dma_start(out=xb[:, dc:dc + 1],
                                          in_=xt[128 * dc:128 * dc + 128, 0:512])
                else:
                    xb = xpool.tile([128, 8, 512], F16, tag="xb")
                    for dc in range(2):
                        nc.sync.dma_start(
                            out=xb[:, 4 * dc:4 * dc + 4],
                            in_=xt[512 * dc:512 * dc + 512, s0:s0 + 512]
                            .rearrange("(i p) s -> p i s", p=128),
                        )
                if blk == 0:
                    b_sb, mask_sb, ones_sb, iden_sb = small_consts()

                # ---- K^T / V^T projections for this block ----
                for name, dst in (("k", kT_sb), ("v", vT_sb)):
                    pp = ps_proj_pool.tile([DK, 512], F32, tag="pp")
                    for dc in range(8):
                        nc.tensor.matmul(
                            pp[:], w_sb[name][:, dc], xb[:, dc],
                            start=(dc == 0), stop=(dc == 7),
                        )
                    nc.vector.tensor_scalar_add(
                        dst[:, s0:s0 + 512], pp[:], b_sb[name][:],
                    )

                # ---- Q^T projection: first 64 cols of each 128-tile ----
                pq = ps_proj_pool.tile([DK, 256], F32, tag="pp")
                for dc in range(8):
                    qmov = xb[:, dc].rearrange("p (t j) -> p t j", t=4)[:, :, 0:64]
                    nc.tensor.matmul(
                        pq[:], w_sb["q"][:, dc], qmov,
                        start=(dc == 0), stop=(dc == 7),
                    )
                q0 = blk * 256
                nc.vector.tensor_scalar_add(qT_sb[:, q0:q0 + 256], pq[:], b_sb["q"][:])

                # ---- V natural tiles (transpose V^T chunks) ----
                tp4 = ps_proj_pool.tile([128, 4, 128], F16, tag="pp")
                for t in range(4):
                    c = 4 * blk + t
                    nc.tensor.matmul(
                        tp4[:, t], vT_sb[:, 128 * c:128 * c + 128], iden_sb[:],
                        is_transpose=True, start=(t == 0), stop=(t == 3),
                    )
                nc.vector.tensor_copy(vnat_sb[:, 4 * blk:4 * blk + 4], tp4[:])

                # ---- attention chunks for this block ----
                # last block reversed: the big chunk 15 goes first so the
                # final chunk on the critical path is the smaller chunk 12;
                # chunk 12's pieces run high-half first so the output's
                # [512:1024] half can normalize + DMA while the PE finishes
                # the [0:512) piece.
                order = range(3, -1, -1) if blk == NBLK - 1 else range(4)
                for t in order:
                    c = 4 * blk + t
                    prefix = 64 * (c + 1)
                    dcol = 64 * c  # diagonal columns [dcol, dcol+64)
                    pieces = [(p, min(512, prefix - p))
                              for p in range(0, prefix, 512)]
                    if c == 12:
                        pieces.reverse()
                    kT_c = kT_sb[:, 128 * c:128 * c + 128]

                    def norm_store(lo, hi):
                        # this column range of the output is final:
                        # normalize + store it while the PE moves on.
                        # ps_sums holds the denominators replicated across
                        # all 128 partitions, so this is pure elementwise
                        # work; the fully-masked column is 0 -> NaN and
                        # the host overwrites it.
                        recip = opool.tile([128, hi - lo], F32,
                                           tag=f"recip{lo}")
                        nc.vector.reciprocal_approx_fast(
                            recip[:], ps_sums[:, lo:hi])
                        o_h = opool.tile([DK, hi - lo], F16, tag=f"o{lo}")
                        nc.vector.tensor_tensor(
                            o_h[:], ps_out[:, lo:hi],
                            recip[:], mybir.AluOpType.mult)
                        nc.sync.dma_start(out=outT[:, lo:hi], in_=o_h[:])
                    # all scores matmuls first (back-to-back on the PE), so
                    # ScalarE runs the exps back-to-back too and exp(p2)
                    # overlaps AV/sums(p1) instead of trailing them
                    pts = []
                    for (p0, pn) in pieces:
                        sc = ps_score_pool.tile([128, 512], F32, tag="sc")
                        nc.tensor.matmul(
                            sc[:, 0:pn], kT_c, qT_sb[:, p0:p0 + pn],
                            start=True, stop=True,
                        )
                        if p0 <= dcol < p0 + pn:
                            dl = dcol - p0
                            nc.vector.tensor_tensor(
                                sc[:, dl:dl + 64], sc[:, dl:dl + 64],
                                mask_sb[:], mybir.AluOpType.add,
                            )
                        pt = ppool.tile([128, 512], F16, tag="pt")
                        nc.scalar.activation(pt[:, 0:pn], sc[:, 0:pn], AF.Exp)
                        pts.append(pt)
                    if c == 12:
                        # chunks 15, 14, 13 are done and chunk 12 only
                        # touches [0, 832): columns [832:1024) are final.
                        # Emitted after chunk 12's mask add so the DVE work
                        # here doesn't delay its exp.
                        norm_store(832, 1024)
                    for (p0, pn), pt in zip(pieces, pts):
                        # the accumulators were DVE-zeroed once up front, so
                        # every matmul accumulates (start=False)
                        nc.tensor.matmul(
                            ps_out[:, p0:p0 + pn], vnat_sb[:, c],
                            pt[:, 0:pn], start=False, stop=False,
                        )
                        nc.tensor.matmul(
                            ps_sums[:, p0:p0 + pn], ones_sb[:],
                            pt[:, 0:pn], start=False, stop=False,
                        )
                        if c == 12:
                            if p0 == 512:
                                norm_store(512, 832)
                            else:
                                norm_store(0, 256)
                                norm_store(256, 512)

    nc.compile()
    return nc


def _prep_inputs(inputs, Wq, bq, Wk, bk, Wv, bv):
    scale = np.float32(1.0 / np.sqrt(DK))
    wq_s = np.ascontiguousarray((Wq * scale).reshape(8, 128, DK).transpose(1, 0, 2)).astype(np.float16)
    wk_s = np.ascontiguousarray(Wk.reshape(8, 128, DK).transpose(1, 0, 2)).astype(np.float16)
    wv_s = np.ascontiguousarray(Wv.reshape(8, 128, DK).transpose(1, 0, 2)).astype(np.float16)
    bq_s = np.ascontiguousarray((bq * scale).reshape(DK, 1), dtype=np.float32)
    bk_s = np.ascontiguousarray(bk.reshape(DK, 1), dtype=np.float32)
    bv_s = np.ascontiguousarray(bv.reshape(DK, 1), dtype=np.float32)
    ones = np.ones((128, 128), dtype=np.float16)
    iden = np.eye(128, dtype=np.float16)

    p = np.arange(128)[:, None]
    j = np.arange(64)[None, :]
    masks = []
    for h in (0, 1):
        m = np.zeros((128, 64), dtype=np.float32)
        m[(p < 64) & (p <= j)] = NEG
        if h == 1:
            m[p[:, 0] >= 64, :] = NEG
        masks.append(m)

    in_maps = []
    for core in range(NCORES):
        b, h = core // 2, core % 2
        xt = inputs[b].T.reshape(D, 16, 2, 64)
        if h == 1:
            xt = xt[:, :, ::-1, :]
        xt = np.ascontiguousarray(xt).reshape(D, S).astype(np.float16)
        in_maps.append({
            "xt": xt, "wq": wq_s, "wk": wk_s, "wv": wv_s,
            "bq": bq_s, "bk": bk_s, "bv": bv_s,
            "maskd": masks[h], "onesd": ones, "idend": iden,
        })
    return in_maps


def kernel(inputs, Wq, bq, Wk, bk, Wv, bv):
    inputs = np.asarray(inputs, dtype=np.float32)
    Wq, bq = np.asarray(Wq), np.asarray(bq)
    Wk, bk = np.asarray(Wk), np.asarray(bk)
    Wv, bv = np.asarray(Wv), np.asarray(bv)
    if "nc" not in _cache:
        _cache["nc"] = _build()
    nc = _cache["nc"]
    in_maps = _prep_inputs(inputs, Wq, bq, Wk, bk, Wv, bv)
    res = run_bass_kernel_spmd(nc, in_maps, list(range(NCORES)))
    out = np.empty((B, S, DK), dtype=np.float32)
    for core in range(NCORES):
        b, h = core // 2, core % 2
        oT = res.results[core]["outT"].astype(np.float32)  # [DK, 1024]
        o = oT.T.reshape(16, 64, DK)            # [c, j, DK]
        out[b].reshape(16, 2, 64, DK)[:, h] = o
    # Fully-masked last row: softmax of a uniform -1e9 row is exactly
    # uniform, so out[2047] = mean_s(V) = mean_s(X) @ Wv + bv.
    xmean = inputs.mean(axis=1)                 # [B, D]
    out[:, S - 1, :] = xmean @ Wv + bv
    return out


# revision 37
# speedup vs baseline: 1.0743x; 1.0020x over previous
"""Masked self-attention Trainium2 kernel (8 NeuronCores, Bass/Tile).

Problem: B=4, S=2048, D=1024, DK=128 fp32.
  Q = X@Wq + bq; K = X@Wk + bk; V = X@Wv + bv
  scores = Q@K^T / sqrt(DK); masked = scores + tril(ones)*(-1e9)
  out = softmax(masked) @ V

Sharding: core = (batch b = core//2) x (row-half h = core%2). Each core
computes 64 query rows of each of the 16 query tiles of its batch
(rows 128c + 64h + j). All cores run an identical program; per-core
differences are carried entirely in the input data (a column
permutation of X^T and a small mask block).

Device layouts (all transposed so the PE contracts over partitions):
  X^T [D, S] (host-transposed, per-tile column permuted: own rows first)
  Q^T/K^T [DK, *] = W-chunks(lhsT) x X^T(moving) matmuls
  scores^T [s-chunk 128, q-prefix] = K^T-chunk(lhsT) x Q^T(moving)
  causal skip: chunk c only attends query tiles qi <= c -> contiguous
  q-prefix of width 64*(c+1); single [128,64] mask block on the last
  64 columns (the diagonal tile)
  softmax: exp without max-subtraction (scores are O(1); masked lanes
  underflow to exactly 0); row sums via an all-ones [128,128] stationary
  matmul that lands the sums REPLICATED across all 128 partitions (same
  PE cost as an M=1 matmul -- cost is moving columns), so normalization
  is a single DVE fast-reciprocal + multiply. No ScalarE ln/exp tail,
  no activation-table switches (Exp is the only table ever loaded).
  out^T [DK, 1024] accumulated in PSUM across s-chunks.
  The globally fully-masked last row (2047) is patched on the HOST:
  softmax of a uniform -1e9 row is exactly uniform, so that row equals
  mean_s(V) = mean_s(X) @ Wv + bv -- a [D]x[D,DK] matvec on numpy.
  (On-device that column is 0/0 -> NaN and is simply overwritten.)

  All matmul operands are float16 (11-bit mantissa, ~2.4e-4 rounding --
  the same precision class as the PE's f32r/TF32 mode for this N(0,1)
  data) with fp32 PSUM accumulation. vs f32r this halves the X DMA,
  enables fast weight loads (FWL; fp32-path LDWEIGHTS cannot be
  hidden), and has no small-N throughput penalty. Range is safe: all
  fp16-stored tensors are O(1)..O(100); scores/sums/outputs stay fp32.
  The first weight chunk gets a dedicated small first-wave DMA because
  the DGE queues fair-share HBM bandwidth and gate the first matmul;
  block 0's X arrives in fine-grained pieces (dc0 first) so the PE
  starts as soon as the first 192 KiB have landed.
"""

import numpy as np

import concourse.bacc as bacc
import concourse.tile as tile
import concourse.mybir as mybir
from concourse.bass_utils import run_bass_kernel_spmd

F32 = mybir.dt.float32
F16 = mybir.dt.float16
AF = mybir.ActivationFunctionType

B, S, D, DK = 4, 2048, 1024, 128
NEG = -1.0e9
NCORES = 8
NBLK = 4          # s-blocks of 512
NCHUNK = 16       # s-chunks of 128
QL = 1024         # local query columns per core (16 tiles x 64)

_cache = {}


def _build():
    nc = bacc.Bacc("TRN2", target_bir_lowering=False, debug=False,
                   num_devices=NCORES)

    xt = nc.dram_tensor("xt", [D, S], F16, kind="ExternalInput")
    wq = nc.dram_tensor("wq", [128, 8, DK], F16, kind="ExternalInput")
    wk = nc.dram_tensor("wk", [128, 8, DK], F16, kind="ExternalInput")
    wv = nc.dram_tensor("wv", [128, 8, DK], F16, kind="ExternalInput")
    bq = nc.dram_tensor("bq", [DK, 1], F32, kind="ExternalInput")
    bk = nc.dram_tensor("bk", [DK, 1], F32, kind="ExternalInput")
    bv = nc.dram_tensor("bv", [DK, 1], F32, kind="ExternalInput")
    maskd = nc.dram_tensor("maskd", [128, 64], F32, kind="ExternalInput")
    onesd = nc.dram_tensor("onesd", [128, 128], F16, kind="ExternalInput")
    idend = nc.dram_tensor("idend", [128, 128], F16, kind="ExternalInput")
    outT = nc.dram_tensor("outT", [DK, QL], F16, kind="ExternalOutput")

    with tile.TileContext(nc) as tc:
        with (
            tc.tile_pool(name="consts", bufs=1) as cpool,
            tc.tile_pool(name="xblk", bufs=3) as xpool,
            tc.tile_pool(name="kv", bufs=1) as kvpool,
            tc.tile_pool(name="pt", bufs=3) as ppool,
            tc.tile_pool(name="outp", bufs=1) as opool,
            tc.tile_pool(name="ps_out", bufs=1, space="PSUM") as ps_out_pool,
            tc.tile_pool(name="ps_sums", bufs=1, space="PSUM") as ps_sums_pool,
            tc.tile_pool(name="ps_proj", bufs=2, space="PSUM") as ps_proj_pool,
            tc.tile_pool(name="ps_score", bufs=2, space="PSUM") as ps_score_pool,
        ):
            # ---- weights first (needed by the very first matmul).
            # The first proj matmul (K, dc=0) gates the whole PE stream, so
            # its 64 KiB weight chunk gets a dedicated first DMA: the DGE
            # queues fair-share HBM bandwidth, so a small exclusive first
            # wave completes ~10x sooner than one queued with everything.
            # Weights ride the scalar ring in need-order (wk, wv, wq) so
            # they serialize behind each other instead of splitting HBM
            # bandwidth with X block 0; two X pieces are interleaved into
            # the ring so X keeps a larger share of the early window.
            w_sb = {}
            xb0 = xpool.tile([128, 8, 512], F16, tag="xb")
            for name, dram in (("k", wk), ("v", wv), ("q", wq)):
                t = cpool.tile([128, 8, DK], F16, tag=f"w{name}")
                if name == "k":
                    nc.scalar.dma_start(out=t[:, 0:1], in_=dram[:, 0:1])
                    nc.scalar.dma_start(out=xb0[:, 1:2], in_=xt[128:256, 0:512])
                    nc.scalar.dma_start(out=t[:, 1:8], in_=dram[:, 1:8])
                    nc.scalar.dma_start(out=xb0[:, 3:4], in_=xt[384:512, 0:512])
                else:
                    nc.scalar.dma_start(out=t[:], in_=dram[:])
                w_sb[name] = t

            def small_consts():
                b_sb = {}
                for name, dram in (("q", bq), ("k", bk), ("v", bv)):
                    t = cpool.tile([DK, 1], F32, tag=f"b{name}")
                    nc.gpsimd.dma_start(out=t[:], in_=dram[:])
                    b_sb[name] = t
                mask_sb = cpool.tile([128, 64], F32, tag="mask")
                nc.gpsimd.dma_start(out=mask_sb[:], in_=maskd[:])
                ones_sb = cpool.tile([128, 128], F16, tag="ones")
                nc.gpsimd.dma_start(out=ones_sb[:], in_=onesd[:])
                iden_sb = cpool.tile([128, 128], F16, tag="iden")
                nc.gpsimd.dma_start(out=iden_sb[:], in_=idend[:])
                return b_sb, mask_sb, ones_sb, iden_sb

            # ---- persistent buffers ----
            kT_sb = kvpool.tile([DK, S], F16, tag="kT")
            qT_sb = kvpool.tile([DK, QL], F16, tag="qT")
            vT_sb = kvpool.tile([DK, S], F16, tag="vT")
            vnat_sb = kvpool.tile([128, NCHUNK, DK], F16, tag="vnat")

            ps_out = ps_out_pool.tile([DK, QL], F32)       # 2 banks
            ps_sums = ps_sums_pool.tile([128, QL], F32)    # 2 banks
            nc.vector.memset(ps_out[:], 0.0)
            nc.vector.memset(ps_sums[:], 0.0)

            for blk in range(NBLK):
                s0 = blk * 512
                # ---- stream X^T block: 8 d-chunk tiles x 512 s-cols ----
                # Block 0 lands in fine pieces (dc0 alone first: together
                # with the wk first wave only ~192 KiB gate the first
                # matmul); later blocks use coarse 2-chunk DMAs to save
                # issue slots on the queues.
                if blk == 0:
                    # single-d-chunk pieces: the PE stalls on each 128 KiB
                    # piece individually instead of a 256 KiB pair while
                    # the early phase is HBM-bound (dc1/dc3 already ride
                    # the scalar ring between the weight DMAs)
                    xb = xb0
                    for dc in (0, 2, 4, 5, 6, 7):
                        nc.sync.dma_start(out=xb[:, dc:dc + 1],
                                          in_=xt[128 * dc:128 * dc + 128, 0:512])
                else:
                    xb = xpool.tile([128, 8, 512], F16, tag="xb")
                    for dc in range(2):
                        nc.sync.dma_start(
                            out=xb[:, 4 * dc:4 * dc + 4],
                            in_=xt[512 * dc:512 * dc + 512, s0:s0 + 512]
                            .rearrange("(i p) s -> p i s", p=128),
                        )
                if blk == 0:
                    b_sb, mask_sb, ones_sb, iden_sb = small_consts()

                # ---- K^T / V^T projections, interleaved per d-chunk ----
                # Each arrived X piece immediately feeds BOTH projections
                # (2x512 cols of PE work per 128 KiB), matching the PE's
                # consumption rate to the HBM delivery rate during the
                # DMA-bound early phase. The two accumulation groups live
                # in separate PSUM banks, so interleaving is safe on HW.
                ppK = ps_proj_pool.tile([DK, 512], F32, tag="pp")
                ppV = ps_proj_pool.tile([DK, 512], F32, tag="pp")
                for dc in range(8):
                    nc.tensor.matmul(
                        ppK[:], w_sb["k"][:, dc], xb[:, dc],
                        start=(dc == 0), stop=(dc == 7),
                        skip_group_check=True,
                    )
                    nc.tensor.matmul(
                        ppV[:], w_sb["v"][:, dc], xb[:, dc],
                        start=(dc == 0), stop=(dc == 7),
                        skip_group_check=True,
                    )
                nc.vector.tensor_scalar_add(
                    kT_sb[:, s0:s0 + 512], ppK[:], b_sb["k"][:],
                )
                nc.vector.tensor_scalar_add(
                    vT_sb[:, s0:s0 + 512], ppV[:], b_sb["v"][:],
                )

                # ---- Q^T projection: first 64 cols of each 128-tile ----
                # pq borrows a score-pool bank: by the time the PE reaches
                # it, the previous block's attention scores are retired, so
                # this frees the proj pool for the V transposes instead of
                # serializing behind the K bias-add.
                pq = ps_score_pool.tile([DK, 256], F32, tag="sc")
                for dc in range(8):
                    qmov = xb[:, dc].rearrange("p (t j) -> p t j", t=4)[:, :, 0:64]
                    nc.tensor.matmul(
                        pq[:], w_sb["q"][:, dc], qmov,
                        start=(dc == 0), stop=(dc == 7),
                    )
                q0 = blk * 256
                nc.vector.tensor_scalar_add(qT_sb[:, q0:q0 + 256], pq[:], b_sb["q"][:])

                # ---- V natural tiles (transpose V^T chunks) ----
                tp4 = ps_proj_pool.tile([128, 4, 128], F16, tag="pp")
                for t in range(4):
                    c = 4 * blk + t
                    nc.tensor.matmul(
                        tp4[:, t], vT_sb[:, 128 * c:128 * c + 128], iden_sb[:],
                        is_transpose=True, start=(t == 0), stop=(t == 3),
                    )
                nc.vector.tensor_copy(vnat_sb[:, 4 * blk:4 * blk + 4], tp4[:])

                # ---- attention for this block: flat piece pipeline ----
                # Last block reversed: the big chunk 15 goes first so the
                # final chunk on the critical path is the smaller chunk 12;
                # chunk 12's pieces run high-half first (last piece split
                # into 2x256) so output ranges normalize + DMA while the
                # PE finishes the rest.
                def norm_store(lo, hi, eng):
                    # this column range of the output is final: normalize
                    # + store it while the PE moves on. ps_sums holds the
                    # denominators replicated across all 128 partitions,
                    # so this is pure elementwise work; the fully-masked
                    # column is 0 -> NaN and the host overwrites it.
                    recip = opool.tile([128, hi - lo], F32, tag=f"recip{lo}")
                    nc.vector.reciprocal_approx_fast(
                        recip[:], ps_sums[:, lo:hi])
                    o_h = opool.tile([DK, hi - lo], F16, tag=f"o{lo}")
                    nc.vector.tensor_tensor(
                        o_h[:], ps_out[:, lo:hi],
                        recip[:], mybir.AluOpType.mult)
                    eng.dma_start(out=outT[:, lo:hi], in_=o_h[:])

                # flat list of (c, p0, pn, norm-after-this-piece)
                flat = []
                order = range(3, -1, -1) if blk == NBLK - 1 else range(4)
                for t in order:
                    c = 4 * blk + t
                    prefix = 64 * (c + 1)
                    if c == 12:
                        # [832:1024) is final once chunks 15/14/13 are in;
                        # [512:832) lands with the first piece below
                        flat.append((c, 512, 320, (512, 1024, nc.sync)))
                        flat.append((c, 256, 256, (256, 512, nc.gpsimd)))
                        flat.append((c, 0, 256, (0, 256, nc.sync)))
                    else:
                        for p in range(0, prefix, 512):
                            flat.append((c, p, min(512, prefix - p), None))

                # software pipeline, lookahead 1: scores(i+1) is emitted
                # between AV/sums(i) so the PE never sits waiting for an
                # exp; sc(i+1) reuses the PSUM bank freed by exp(i-1).
                def front(c, p0, pn, norm):
                    dcol = 64 * c
                    sc = ps_score_pool.tile([128, 512], F32, tag="sc")
                    nc.tensor.matmul(
                        sc[:, 0:pn], kT_sb[:, 128 * c:128 * c + 128],
                        qT_sb[:, p0:p0 + pn], start=True, stop=True,
                    )
                    if p0 <= dcol < p0 + pn:
                        dl = dcol - p0
                        nc.vector.tensor_tensor(
                            sc[:, dl:dl + 64], sc[:, dl:dl + 64],
                            mask_sb[:], mybir.AluOpType.add,
                        )
                    pt = ppool.tile([128, 512], F16, tag="pt")
                    nc.scalar.activation(pt[:, 0:pn], sc[:, 0:pn], AF.Exp)
                    return (c, p0, pn, norm, pt)

                def back(c, p0, pn, norm, pt):
                    # the accumulators were DVE-zeroed once up front, so
                    # every matmul accumulates (start=False)
                    nc.tensor.matmul(
                        ps_out[:, p0:p0 + pn], vnat_sb[:, c],
                        pt[:, 0:pn], start=False, stop=False,
                    )
                    nc.tensor.matmul(
                        ps_sums[:, p0:p0 + pn], ones_sb[:],
                        pt[:, 0:pn], start=False, stop=False,
                    )
                    if norm is not None:
                        norm_store(*norm)

                inflight = []
                for piece in flat:
                    inflight.append(front(*piece))
                    if len(inflight) >= 2:
                        back(*inflight.pop(0))
                for fr in inflight:
                    back(*fr)

    nc.compile()
    return nc


def _prep_inputs(inputs, Wq, bq, Wk, bk, Wv, bv):
    scale = np.float32(1.0 / np.sqrt(DK))
    wq_s = np.ascontiguousarray((Wq * scale).reshape(8, 128, DK).transpose(1, 0, 2)).astype(np.float16)
    wk_s = np.ascontiguousarray(Wk.reshape(8, 128, DK).transpose(1, 0, 2)).astype(np.float16)
    wv_s = np.ascontiguousarray(Wv.reshape(8, 128, DK).transpose(1, 0, 2)).astype(np.float16)
    bq_s = np.ascontiguousarray((bq * scale).reshape(DK, 1), dtype=np.float32)
    bk_s = np.ascontiguousarray(bk.reshape(DK, 1), dtype=np.float32)
    bv_s = np.ascontiguousarray(bv.reshape(DK, 1), dtype=np.float32)
    ones = np.ones((128, 128), dtype=np.float16)
    iden = np.eye(128, dtype=np.float16)

    p = np.arange(128)[:, None]
    j = np.arange(64)[None, :]
    masks = []
    for h in (0, 1):
        m = np.zeros((128, 64), dtype=np.float32)
        m[(p < 64) & (p <= j)] = NEG
        if h == 1:
            m[p[:, 0] >= 64, :] = NEG
        masks.append(m)

    in_maps = []
    for core in range(NCORES):
        b, h = core // 2, core % 2
        xt = inputs[b].T.reshape(D, 16, 2, 64)
        if h == 1:
            xt = xt[:, :, ::-1, :]
        xt = np.ascontiguousarray(xt).reshape(D, S).astype(np.float16)
        in_maps.append({
            "xt": xt, "wq": wq_s, "wk": wk_s, "wv": wv_s,
            "bq": bq_s, "bk": bk_s, "bv": bv_s,
            "maskd": masks[h], "onesd": ones, "idend": iden,
        })
    return in_maps


def kernel(inputs, Wq, bq, Wk, bk, Wv, bv):
    inputs = np.asarray(inputs, dtype=np.float32)
    Wq, bq = np.asarray(Wq), np.asarray(bq)
    Wk, bk = np.asarray(Wk), np.asarray(bk)
    Wv, bv = np.asarray(Wv), np.asarray(bv)
    if "nc" not in _cache:
        _cache["nc"] = _build()
    nc = _cache["nc"]
    in_maps = _prep_inputs(inputs, Wq, bq, Wk, bk, Wv, bv)
    res = run_bass_kernel_spmd(nc, in_maps, list(range(NCORES)))
    out = np.empty((B, S, DK), dtype=np.float32)
    for core in range(NCORES):
        b, h = core // 2, core % 2
        oT = res.results[core]["outT"].astype(np.float32)  # [DK, 1024]
        o = oT.T.reshape(16, 64, DK)            # [c, j, DK]
        out[b].reshape(16, 2, 64, DK)[:, h] = o
    # Fully-masked last row: softmax of a uniform -1e9 row is exactly
    # uniform, so out[2047] = mean_s(V) = mean_s(X) @ Wv + bv.
    xmean = inputs.mean(axis=1)                 # [B, D]
    out[:, S - 1, :] = xmean @ Wv + bv
    return out


# revision 43
# speedup vs baseline: 1.0852x; 1.0101x over previous
"""Masked self-attention Trainium2 kernel (8 NeuronCores, Bass/Tile).

Problem: B=4, S=2048, D=1024, DK=128 fp32.
  Q = X@Wq + bq; K = X@Wk + bk; V = X@Wv + bv
  scores = Q@K^T / sqrt(DK); masked = scores + tril(ones)*(-1e9)
  out = softmax(masked) @ V

Sharding: core = (batch b = core//2) x (row-half h = core%2). Each core
computes 64 query rows of each of the 16 query tiles of its batch
(rows 128c + 64h + j). All cores run an identical program; per-core
differences are carried entirely in the input data (a column
permutation of X^T and a small mask block).

Device layouts (all transposed so the PE contracts over partitions):
  X^T [D, S] (host-transposed, per-tile column permuted: own rows first)
  Q^T/K^T [DK, *] = W-chunks(lhsT) x X^T(moving) matmuls
  scores^T [s-chunk 128, q-prefix] = K^T-chunk(lhsT) x Q^T(moving)
  causal skip: chunk c only attends query tiles qi <= c -> contiguous
  q-prefix of width 64*(c+1); single [128,64] mask block on the last
  64 columns (the diagonal tile)
  softmax: exp without max-subtraction (scores are O(1); masked lanes
  underflow to exactly 0); row sums via an all-ones [128,128] stationary
  matmul that lands the sums REPLICATED across all 128 partitions (same
  PE cost as an M=1 matmul -- cost is moving columns), so normalization
  is a single DVE fast-reciprocal + multiply. No ScalarE ln/exp tail,
  no activation-table switches (Exp is the only table ever loaded).
  out^T [DK, 1024] accumulated in PSUM across s-chunks.
  The globally fully-masked last row (2047) is patched on the HOST:
  softmax of a uniform -1e9 row is exactly uniform, so that row equals
  mean_s(V) = mean_s(X) @ Wv + bv -- a [D]x[D,DK] matvec on numpy.
  (On-device that column is 0/0 -> NaN and is simply overwritten.)

  All matmul operands are float16 (11-bit mantissa, ~2.4e-4 rounding --
  the same precision class as the PE's f32r/TF32 mode for this N(0,1)
  data) with fp32 PSUM accumulation. vs f32r this halves the X DMA,
  enables fast weight loads (FWL; fp32-path LDWEIGHTS cannot be
  hidden), and has no small-N throughput penalty. Range is safe: all
  fp16-stored tensors are O(1)..O(100); scores/sums/outputs stay fp32.
  The first weight chunk gets a dedicated small first-wave DMA because
  the DGE queues fair-share HBM bandwidth and gate the first matmul;
  block 0's X arrives in fine-grained pieces (dc0 first) so the PE
  starts as soon as the first 192 KiB have landed.
"""

import numpy as np

import concourse.bacc as bacc
import concourse.tile as tile
import concourse.mybir as mybir
from concourse.bass_utils import run_bass_kernel_spmd

F32 = mybir.dt.float32
F16 = mybir.dt.float16
AF = mybir.ActivationFunctionType

B, S, D, DK = 4, 2048, 1024, 128
NEG = -1.0e9
NCORES = 8
NBLK = 4          # s-blocks of 512
NCHUNK = 16       # s-chunks of 128
QL = 1024         # local query columns per core (16 tiles x 64)

_cache = {}


def _build():
    nc = bacc.Bacc("TRN2", target_bir_lowering=False, debug=False,
                   num_devices=NCORES)

    xt = nc.dram_tensor("xt", [D, S], F16, kind="ExternalInput")
    wq = nc.dram_tensor("wq", [128, 8, DK], F16, kind="ExternalInput")
    wk = nc.dram_tensor("wk", [128, 8, DK], F16, kind="ExternalInput")
    wv = nc.dram_tensor("wv", [128, 8, DK], F16, kind="ExternalInput")
    bq = nc.dram_tensor("bq", [DK, 1], F32, kind="ExternalInput")
    bk = nc.dram_tensor("bk", [DK, 1], F32, kind="ExternalInput")
    bv = nc.dram_tensor("bv", [DK, 1], F32, kind="ExternalInput")
    maskd = nc.dram_tensor("maskd", [128, 64], F32, kind="ExternalInput")
    onesd = nc.dram_tensor("onesd", [128, 128], F16, kind="ExternalInput")
    idend = nc.dram_tensor("idend", [128, 128], F16, kind="ExternalInput")
    outT = nc.dram_tensor("outT", [DK, QL], F16, kind="ExternalOutput")

    with tile.TileContext(nc) as tc:
        with (
            tc.tile_pool(name="consts", bufs=1) as cpool,
            tc.tile_pool(name="xblk", bufs=3) as xpool,
            tc.tile_pool(name="kv", bufs=1) as kvpool,
            tc.tile_pool(name="pt", bufs=3) as ppool,
            tc.tile_pool(name="outp", bufs=1) as opool,
            tc.tile_pool(name="ps_out", bufs=1, space="PSUM") as ps_out_pool,
            tc.tile_pool(name="ps_sums", bufs=1, space="PSUM") as ps_sums_pool,
            # one merged 4-buffer pool (4 banks) for proj AND score tiles:
            # during projection it holds ppK/ppV/pq/tp4; during attention
            # all 4 buffers rotate score tiles, giving the piece pipeline
            # a 4-deep rotation (depth 2 made scores wait for exps)
            tc.tile_pool(name="ps_work", bufs=4, space="PSUM") as ps_work_pool,
        ):
            # ---- weights first (needed by the very first matmul).
            # The first proj matmul (K, dc=0) gates the whole PE stream, so
            # its 64 KiB weight chunk gets a dedicated first DMA: the DGE
            # queues fair-share HBM bandwidth, so a small exclusive first
            # wave completes ~10x sooner than one queued with everything.
            # Weights ride the scalar ring with wk and wv interleaved in
            # per-d-chunk-group need-order: the K/V projections consume
            # both weights per d-chunk, so wv chunk 0 must land right
            # after wk chunk 0, not behind all of wk.
            w_sb = {}
            xb0 = xpool.tile([128, 8, 512], F16, tag="xb")
            for name in ("k", "v", "q"):
                t = cpool.tile([128, 8, DK], F16, tag=f"w{name}")
                w_sb[name] = t
            nc.scalar.dma_start(out=w_sb["k"][:, 0:1], in_=wk[:, 0:1])
            nc.scalar.dma_start(out=w_sb["v"][:, 0:1], in_=wv[:, 0:1])
            nc.scalar.dma_start(out=w_sb["k"][:, 1:4], in_=wk[:, 1:4])
            nc.scalar.dma_start(out=w_sb["v"][:, 1:4], in_=wv[:, 1:4])
            nc.scalar.dma_start(out=w_sb["k"][:, 4:8], in_=wk[:, 4:8])
            nc.scalar.dma_start(out=w_sb["v"][:, 4:8], in_=wv[:, 4:8])
            nc.scalar.dma_start(out=w_sb["q"][:], in_=wq[:])

            def small_consts():
                b_sb = {}
                for name, dram in (("q", bq), ("k", bk), ("v", bv)):
                    t = cpool.tile([DK, 1], F32, tag=f"b{name}")
                    nc.gpsimd.dma_start(out=t[:], in_=dram[:])
                    b_sb[name] = t
                mask_sb = cpool.tile([128, 64], F32, tag="mask")
                nc.gpsimd.dma_start(out=mask_sb[:], in_=maskd[:])
                ones_sb = cpool.tile([128, 128], F16, tag="ones")
                nc.gpsimd.dma_start(out=ones_sb[:], in_=onesd[:])
                iden_sb = cpool.tile([128, 128], F16, tag="iden")
                nc.gpsimd.dma_start(out=iden_sb[:], in_=idend[:])
                return b_sb, mask_sb, ones_sb, iden_sb

            # ---- persistent buffers ----
            kT_sb = kvpool.tile([DK, S], F16, tag="kT")
            qT_sb = kvpool.tile([DK, QL], F16, tag="qT")
            vT_sb = kvpool.tile([DK, S], F16, tag="vT")
            vnat_sb = kvpool.tile([128, NCHUNK, DK], F16, tag="vnat")

            ps_out = ps_out_pool.tile([DK, QL], F32)       # 2 banks
            ps_sums = ps_sums_pool.tile([128, QL], F32)    # 2 banks
            nc.vector.memset(ps_out[:], 0.0)
            nc.vector.memset(ps_sums[:], 0.0)

            for blk in range(NBLK):
                s0 = blk * 512
                # ---- stream X^T block: 8 d-chunk tiles x 512 s-cols ----
                # Block 0 lands in fine pieces (dc0 alone first: together
                # with the wk first wave only ~192 KiB gate the first
                # matmul); later blocks use coarse 2-chunk DMAs to save
                # issue slots on the queues.
                if blk == 0:
                    # fine-grained pieces: the PE stalls on each piece
                    # individually while the early phase is HBM-bound
                    xb = xb0
                    nc.sync.dma_start(out=xb[:, 0:1], in_=xt[0:128, 0:512])
                    nc.sync.dma_start(out=xb[:, 1:2], in_=xt[128:256, 0:512])
                    for dc in range(1, 4):
                        nc.sync.dma_start(
                            out=xb[:, 2 * dc:2 * dc + 2],
                            in_=xt[256 * dc:256 * dc + 256, 0:512]
                            .rearrange("(i p) s -> p i s", p=128),
                        )
                else:
                    xb = xpool.tile([128, 8, 512], F16, tag="xb")
                    for dc in range(2):
                        nc.sync.dma_start(
                            out=xb[:, 4 * dc:4 * dc + 4],
                            in_=xt[512 * dc:512 * dc + 512, s0:s0 + 512]
                            .rearrange("(i p) s -> p i s", p=128),
                        )
                if blk == 0:
                    b_sb, mask_sb, ones_sb, iden_sb = small_consts()

                # ---- K^T / V^T projections, interleaved per d-chunk ----
                # Each arrived X piece immediately feeds BOTH projections
                # (2x512 cols of PE work per 128 KiB), matching the PE's
                # consumption rate to the HBM delivery rate during the
                # DMA-bound early phase. The two accumulation groups live
                # in separate PSUM banks, so interleaving is safe on HW.
                ppK = ps_work_pool.tile([DK, 512], F32, tag="pp")
                ppV = ps_work_pool.tile([DK, 512], F32, tag="pp")
                for dc in range(8):
                    nc.tensor.matmul(
                        ppK[:], w_sb["k"][:, dc], xb[:, dc],
                        start=(dc == 0), stop=(dc == 7),
                        skip_group_check=True,
                    )
                    nc.tensor.matmul(
                        ppV[:], w_sb["v"][:, dc], xb[:, dc],
                        start=(dc == 0), stop=(dc == 7),
                        skip_group_check=True,
                    )
                nc.vector.tensor_scalar_add(
                    kT_sb[:, s0:s0 + 512], ppK[:], b_sb["k"][:],
                )
                nc.vector.tensor_scalar_add(
                    vT_sb[:, s0:s0 + 512], ppV[:], b_sb["v"][:],
                )

                # ---- Q^T projection: first 64 cols of each 128-tile ----
                # pq borrows a score-pool bank: by the time the PE reaches
                # it, the previous block's attention scores are retired, so
                # this frees the proj pool for the V transposes instead of
                # serializing behind the K bias-add.
                pq = ps_work_pool.tile([DK, 256], F32, tag="pp")
                for dc in range(8):
                    qmov = xb[:, dc].rearrange("p (t j) -> p t j", t=4)[:, :, 0:64]
                    nc.tensor.matmul(
                        pq[:], w_sb["q"][:, dc], qmov,
                        start=(dc == 0), stop=(dc == 7),
                    )
                q0 = blk * 256
                nc.vector.tensor_scalar_add(qT_sb[:, q0:q0 + 256], pq[:], b_sb["q"][:])

                # ---- V natural tiles (transpose V^T chunks) ----
                tp4 = ps_work_pool.tile([128, 4, 128], F16, tag="pp")
                for t in range(4):
                    c = 4 * blk + t
                    nc.tensor.matmul(
                        tp4[:, t], vT_sb[:, 128 * c:128 * c + 128], iden_sb[:],
                        is_transpose=True, start=(t == 0), stop=(t == 3),
                    )
                nc.vector.tensor_copy(vnat_sb[:, 4 * blk:4 * blk + 4], tp4[:])

                # ---- attention for this block: flat piece pipeline ----
                # Last block reversed: the big chunk 15 goes first so the
                # final chunk on the critical path is the smaller chunk 12;
                # chunk 12's pieces run high-half first (last piece split
                # into 2x256) so output ranges normalize + DMA while the
                # PE finishes the rest.
                def norm_store(lo, hi, eng):
                    # this column range of the output is final: normalize
                    # + store it while the PE moves on. ps_sums holds the
                    # denominators replicated across all 128 partitions,
                    # so this is pure elementwise work; the fully-masked
                    # column is 0 -> NaN and the host overwrites it.
                    recip = opool.tile([128, hi - lo], F32, tag=f"recip{lo}")
                    nc.vector.reciprocal_approx_fast(
                        recip[:], ps_sums[:, lo:hi])
                    o_h = opool.tile([DK, hi - lo], F16, tag=f"o{lo}")
                    nc.vector.tensor_tensor(
                        o_h[:], ps_out[:, lo:hi],
                        recip[:], mybir.AluOpType.mult)
                    eng.dma_start(out=outT[:, lo:hi], in_=o_h[:])

                # flat list of (c, p0, pn, norm-after-this-piece)
                flat = []
                order = range(3, -1, -1) if blk == NBLK - 1 else range(4)
                for t in order:
                    c = 4 * blk + t
                    prefix = 64 * (c + 1)
                    if c == 12:
                        # [832:1024) is final once chunks 15/14/13 are in;
                        # [512:832) lands with the first piece below
                        flat.append((c, 512, 320, (512, 1024, nc.sync)))
                        flat.append((c, 256, 256, (256, 512, nc.gpsimd)))
                        flat.append((c, 0, 256, (0, 256, nc.sync)))
                    else:
                        for p in range(0, prefix, 512):
                            flat.append((c, p, min(512, prefix - p), None))

                # software pipeline, lookahead 1: scores(i+1) is emitted
                # between AV/sums(i) so the PE never sits waiting for an
                # exp; sc(i+1) reuses the PSUM bank freed by exp(i-1).
                def front(c, p0, pn, norm):
                    dcol = 64 * c
                    sc = ps_work_pool.tile([128, 512], F32, tag="pp")
                    nc.tensor.matmul(
                        sc[:, 0:pn], kT_sb[:, 128 * c:128 * c + 128],
                        qT_sb[:, p0:p0 + pn], start=True, stop=True,
                    )
                    if p0 <= dcol < p0 + pn:
                        dl = dcol - p0
                        nc.vector.tensor_tensor(
                            sc[:, dl:dl + 64], sc[:, dl:dl + 64],
                            mask_sb[:], mybir.AluOpType.add,
                        )
                    pt = ppool.tile([128, 512], F16, tag="pt")
                    nc.scalar.activation(pt[:, 0:pn], sc[:, 0:pn], AF.Exp)
                    return (c, p0, pn, norm, pt)

                def back(c, p0, pn, norm, pt):
                    # the accumulators were DVE-zeroed once up front, so
                    # every matmul accumulates (start=False)
                    nc.tensor.matmul(
                        ps_out[:, p0:p0 + pn], vnat_sb[:, c],
                        pt[:, 0:pn], start=False, stop=False,
                    )
                    nc.tensor.matmul(
                        ps_sums[:, p0:p0 + pn], ones_sb[:],
                        pt[:, 0:pn], start=False, stop=False,
                    )
                    if norm is not None:
                        norm_store(*norm)

                inflight = []
                for piece in flat:
                    inflight.append(front(*piece))
                    if len(inflight) >= 2:
                        back(*inflight.pop(0))
                for fr in inflight:
                    back(*fr)

    nc.compile()
    return nc


def _prep_inputs(inputs, Wq, bq, Wk, bk, Wv, bv):
    scale = np.float32(1.0 / np.sqrt(DK))
    wq_s = np.ascontiguousarray((Wq * scale).reshape(8, 128, DK).transpose(1, 0, 2)).astype(np.float16)
    wk_s = np.ascontiguousarray(Wk.reshape(8, 128, DK).transpose(1, 0, 2)).astype(np.float16)
    wv_s = np.ascontiguousarray(Wv.reshape(8, 128, DK).transpose(1, 0, 2)).astype(np.float16)
    bq_s = np.ascontiguousarray((bq * scale).reshape(DK, 1), dtype=np.float32)
    bk_s = np.ascontiguousarray(bk.reshape(DK, 1), dtype=np.float32)
    bv_s = np.ascontiguousarray(bv.reshape(DK, 1), dtype=np.float32)
    ones = np.ones((128, 128), dtype=np.float16)
    iden = np.eye(128, dtype=np.float16)

    p = np.arange(128)[:, None]
    j = np.arange(64)[None, :]
    masks = []
    for h in (0, 1):
        m = np.zeros((128, 64), dtype=np.float32)
        m[(p < 64) & (p <= j)] = NEG
        if h == 1:
            m[p[:, 0] >= 64, :] = NEG
        masks.append(m)

    in_maps = []
    for core in range(NCORES):
        b, h = core // 2, core % 2
        xt = inputs[b].T.reshape(D, 16, 2, 64)
        if h == 1:
            xt = xt[:, :, ::-1, :]
        xt = np.ascontiguousarray(xt).reshape(D, S).astype(np.float16)
        in_maps.append({
            "xt": xt, "wq": wq_s, "wk": wk_s, "wv": wv_s,
            "bq": bq_s, "bk": bk_s, "bv": bv_s,
            "maskd": masks[h], "onesd": ones, "idend": iden,
        })
    return in_maps


def kernel(inputs, Wq, bq, Wk, bk, Wv, bv):
    inputs = np.asarray(inputs, dtype=np.float32)
    Wq, bq = np.asarray(Wq), np.asarray(bq)
    Wk, bk = np.asarray(Wk), np.asarray(bk)
    Wv, bv = np.asarray(Wv), np.asarray(bv)
    if "nc" not in _cache:
        _cache["nc"] = _build()
    nc = _cache["nc"]
    in_maps = _prep_inputs(inputs, Wq, bq, Wk, bk, Wv, bv)
    res = run_bass_kernel_spmd(nc, in_maps, list(range(NCORES)))
    out = np.empty((B, S, DK), dtype=np.float32)
    for core in range(NCORES):
        b, h = core // 2, core % 2
        oT = res.results[core]["outT"].astype(np.float32)  # [DK, 1024]
        o = oT.T.reshape(16, 64, DK)            # [c, j, DK]
        out[b].reshape(16, 2, 64, DK)[:, h] = o
    # Fully-masked last row: softmax of a uniform -1e9 row is exactly
    # uniform, so out[2047] = mean_s(V) = mean_s(X) @ Wv + bv.
    xmean = inputs.mean(axis=1)                 # [B, D]
    out[:, S - 1, :] = xmean @ Wv + bv
    return out
